# revision 1
# baseline (speedup 1.0000x reference)
"""Bass/Trainium2 kernel for BiDirectionalSymplecticLayer.

Reference computation (B=8192, T=64, F=128, STEPS=8, DT=0.1):
    q_mid = x[:, 32, :]; p_mid = q_mid - x[:, 31, :]
    H(s) = sum(tanh(tanh(s@W1+b1)@W2+b2) @ Wout),  s = [q, p]  (2F = 256)
    leapfrog forward 4 steps with dt=+0.1 and backward 4 steps with dt=-0.1
    out = concat([q_b, p_b, q_mid, p_mid, q_f, p_f], axis=-1)   # [B, 768]

Device strategy (pure data parallel over 8 cores, 1024 samples each):
  * transposed activations: features on partitions, batch on free dim
  * grad_H(s) = ((((1-h2^2) . wout) @ W2.T) . (1-h1^2)) @ W1.T
  * fold wout into W2T (W2TW[j,i] = wout[j]*W2[i,j]);
    feed sq2 = h2^2 straight into that matmul and correct with the
    column-sum constant c2 using one fused scalar_tensor_tensor:
        v = (psum - c2) * (sq1 - 1) = dh1 * (1 - h1^2)
  * matmul operands in fp16 (full PE rate; fp32 is 4x slower and
    float32r trips a walrus codegen limit); fp32 master states, fp32
    PSUM accumulation, fp32 tanh-derivative path
  * leapfrog scales applied in the state update via one fused
    scalar_tensor_tensor: p' = (g * s) + p
"""

import os
import sys

import numpy as np
import ml_dtypes

try:
    import concourse.bass as bass
except ImportError:  # fresh grading dir: fall back to the repo paths
    for p in ("/root/.axon_site", "/root/.axon_site/_ro/trn_rl_repo",
              "/root/.axon_site/_ro/pypackages", "/opt/trn_rl_repo", "/opt/pypackages"):
        if os.path.isdir(p) and p not in sys.path:
            sys.path.append(p)
    import concourse.bass as bass

import concourse.bacc as bacc
import concourse.mybir as mybir
import concourse.tile as tile
from concourse.bass_utils import run_bass_kernel_spmd

F32 = mybir.dt.float32
F16 = mybir.dt.bfloat16
ALU = mybir.AluOpType
AF = mybir.ActivationFunctionType

N_CORES = 8
B = 8192
Bc = B // N_CORES          # 1024 samples per core
F = 128                    # feature dim (= partition dim)
MID = 32
STEPS_HALF = 4             # leapfrog steps per direction
DT = 0.1
NB = Bc // 512             # matmul moving-dim chunks per tile


def _build_program():
    nc = bacc.Bacc()

    # per-core inputs
    qt_d = nc.declare_dram_parameter("qt", [F, Bc], F32, isOutput=False)
    pt_d = nc.declare_dram_parameter("pt", [F, Bc], F32, isOutput=False)
    qt16_d = nc.declare_dram_parameter("qt16", [F, Bc], F16, isOutput=False)
    pt16_d = nc.declare_dram_parameter("pt16", [F, Bc], F16, isOutput=False)
    # replicated weights (fp16), pre-blocked host-side to [128, kc, 256]
    w1_d = nc.declare_dram_parameter("w1", [F, 2, 2 * F], F16, isOutput=False)
    w2_d = nc.declare_dram_parameter("w2", [F, 2, 2 * F], F16, isOutput=False)
    w2tw_d = nc.declare_dram_parameter("w2tw", [F, 2, 2 * F], F16, isOutput=False)
    w1t_d = nc.declare_dram_parameter("w1t", [F, 2, 2 * F], F16, isOutput=False)
    b1_d = nc.declare_dram_parameter("b1c", [F, 2], F32, isOutput=False)
    b2_d = nc.declare_dram_parameter("b2c", [F, 2], F32, isOutput=False)
    c2_d = nc.declare_dram_parameter("c2c", [F, 2], F32, isOutput=False)
    # per-core outputs (transposed states)
    outs_d = {
        k: nc.declare_dram_parameter(k, [F, Bc], F32, isOutput=True)
        for k in ("oqf", "opf", "oqb", "opb")
    }

    with tile.TileContext(nc) as tc:
        with (
            tc.tile_pool(name="consts", bufs=1) as cw,
            tc.tile_pool(name="states", bufs=2) as stp,
            tc.tile_pool(name="acts", bufs=1) as ap_,
            tc.tile_pool(name="psum", bufs=2, space="PSUM") as pp,
        ):
            w1s = cw.tile([F, 2, 2 * F], F16, name="w1s")
            w2s = cw.tile([F, 2, 2 * F], F16, name="w2s")
            w2tws = cw.tile([F, 2, 2 * F], F16, name="w2tws")
            w1ts = cw.tile([F, 2, 2 * F], F16, name="w1ts")
            b1s = cw.tile([F, 2], F32, name="b1s")
            b2s = cw.tile([F, 2], F32, name="b2s")
            c2s = cw.tile([F, 2], F32, name="c2s")

            # warm the ACT table (tanh/square set) at t=0, hidden under DMAs
            warm = cw.tile([F, 1], F32, name="warm")
            nc.scalar.activation(warm[:], b1s[:, 0:1], AF.Tanh)
            # fp16 initial state (host-cast) feeds the first eval directly
            qh0 = ap_.tile([F, Bc], F16, name="qh_init", tag="qh0")
            ph0 = ap_.tile([F, Bc], F16, name="ph_init", tag="ph0")
            nc.sync.dma_start(out=qh0[:], in_=qt16_d[:])
            nc.sync.dma_start(out=ph0[:], in_=pt16_d[:])
            nc.sync.dma_start(out=w1s[:], in_=w1_d[:])
            nc.sync.dma_start(out=b1s[:], in_=b1_d[:])
            nc.sync.dma_start(out=b2s[:], in_=b2_d[:])
            nc.sync.dma_start(out=c2s[:], in_=c2_d[:])
            nc.sync.dma_start(out=w2s[:], in_=w2_d[:])
            nc.sync.dma_start(out=w2tws[:], in_=w2tw_d[:])
            nc.sync.dma_start(out=w1ts[:], in_=w1t_d[:])

            # chain 0 = forward (dt=+DT), chain 1 = backward (dt=-DT)
            q = [None, None]
            p = [None, None]
            for c in range(2):
                q[c] = stp.tile([F, Bc], F32, name=f"q{c}", tag=f"q{c}")
                p[c] = stp.tile([F, Bc], F32, name=f"p{c}", tag=f"p{c}")
                nc.sync.dma_start(out=q[c][:], in_=qt_d[:])
                nc.sync.dma_start(out=p[c][:], in_=pt_d[:])
            init16 = {"qh": qh0, "ph": ph0}

            def mm_layer(dst2, w, rhs2, jcs=(0, 1)):
                # dst2[jc][m, b] += sum_kc w[:, kc, jc*128+m].T @ rhs2[kc]
                # kc-outer / n-inner: consecutive matmuls share the
                # stationary operand so walrus ldw-opt can elide reloads
                for jc in jcs:
                    for kc in range(2):
                        for n in range(NB):
                            sl = slice(n * 512, (n + 1) * 512)
                            nc.tensor.matmul(
                                dst2[jc][:, sl],
                                w[:, kc, jc * F:(jc + 1) * F],
                                rhs2[kc][:, sl],
                                start=(kc == 0),
                                stop=(kc == 1),
                            )

            def emit_eval(c, first, ev_id, update_chains):
                phase2 = _emit_phase1(c, first, ev_id, update_chains)
                phase2()

            def _emit_phase1(c, first, ev_id, update_chains):
                tg = f"_{c}_{ev_id}"
                # fp16 casts of the current state (GpSimd: slower per-op but
                # otherwise idle; frees DVE cycles)
                if ev_id == 0:
                    qh, ph = init16["qh"], init16["ph"]
                else:
                    qh = ap_.tile([F, Bc], F16, name=f"qh{tg}", tag=f"qh{c}")
                    nc.scalar.activation(qh[:], q[c][:], AF.Copy)
                    ph = ap_.tile([F, Bc], F16, name=f"ph{tg}", tag=f"ph{c}")
                    nc.vector.tensor_copy(ph[:], p[c][:])
                # L1: z1 = [q;p] @ W1
                pz1 = [pp.tile([F, Bc], F32, name=f"pz1{jc}{tg}", tag=f"ps{c}")
                       for jc in range(2)]
                mm_layer(pz1, w1s, (qh, ph))
                h1 = [ap_.tile([F, Bc], F16, name=f"h1{jc}{tg}", tag=f"h1{jc}_{c}")
                      for jc in range(2)]
                for jc in range(2):
                    nc.scalar.activation(h1[jc][:], pz1[jc][:], AF.Tanh,
                                         bias=b1s[:, jc:jc + 1], scale=1.0)
                # sq1 / m1n only need h1 -> emit early (fills ACT/DVE during
                # L2/L3 PE work and shortens the eval tail). Both chunks live
                # in one [F, 2*Bc] tile so m1n is a single DVE op.
                sq1 = ap_.tile([F, 2 * Bc], F16, name=f"sq1{tg}", tag=f"sq1_{c}")
                m1n = ap_.tile([F, 2 * Bc], F16, name=f"m1n{tg}", tag=f"m1n_{c}")
                for jc in range(2):
                    nc.vector.tensor_tensor(sq1[:, jc * Bc:(jc + 1) * Bc],
                                            h1[jc][:], h1[jc][:], ALU.mult)
                nc.vector.tensor_scalar(m1n[:], sq1[:], 1.0, None, ALU.subtract)

                def phase2():
                    _emit_phase2(c, first, ev_id, update_chains, tg, h1, m1n)
                return phase2

            def _emit_phase2(c, first, ev_id, update_chains, tg, h1, m1n):
                # L2: z2 = h1 @ W2
                pz2 = [pp.tile([F, Bc], F32, name=f"pz2{jc}{tg}", tag=f"ps{c}")
                       for jc in range(2)]
                mm_layer(pz2, w2s, h1)
                h2 = [ap_.tile([F, Bc], F32, name=f"h2{jc}{tg}", tag=f"h2{jc}_{c}")
                      for jc in range(2)]
                sq2 = [ap_.tile([F, Bc], F16, name=f"sq2{jc}{tg}", tag=f"sq2{jc}_{c}")
                       for jc in range(2)]
                for jc in range(2):
                    nc.scalar.activation(h2[jc][:], pz2[jc][:], AF.Tanh,
                                         bias=b2s[:, jc:jc + 1], scale=1.0)
                # engine split for balance: chunk0 on ACT; chunk1 mostly ACT
                nc.scalar.activation(sq2[0][:], h2[0][:], AF.Square)
                if ev_id % 2 == 1 and c == 0:
                    nc.vector.tensor_tensor(sq2[1][:], h2[1][:], h2[1][:], ALU.mult)
                else:
                    nc.scalar.activation(sq2[1][:], h2[1][:], AF.Square)
                # L3: pd = sq2 @ W2TW   (true dh1 = c2 - pd)
                pd = [pp.tile([F, Bc], F32, name=f"pd{jc}{tg}", tag=f"ps{c}")
                      for jc in range(2)]
                mm_layer(pd, w2tws, sq2)
                v = [ap_.tile([F, Bc], F16, name=f"v{jc}{tg}", tag=f"v{jc}_{c}")
                     for jc in range(2)]
                for jc in range(2):
                    # v = (pd - c2) * (sq1 - 1) = dh1 * (1 - h1^2)
                    nc.vector.scalar_tensor_tensor(
                        v[jc][:], pd[jc][:], c2s[:, jc:jc + 1],
                        m1n[:, jc * Bc:(jc + 1) * Bc], ALU.subtract, ALU.mult)
                # L4: pg = v @ W1T (unscaled gradient)
                jcs = (0, 1) if first else (0,)
                pg = [pp.tile([F, Bc], F32, name=f"pg{jc}{tg}", tag=f"ps{c}")
                      for jc in jcs]
                mm_layer(pg, w1ts, v, jcs=jcs)
                # state updates: p' = pg*sc_p + p ; q' = pg*sc_q + q
                for uc in update_chains:
                    sc_p = -0.5 * DT if uc == 0 else 0.5 * DT
                    sc_q = DT if uc == 0 else -DT
                    pn = stp.tile([F, Bc], F32, name=f"pn{uc}{tg}", tag=f"p{uc}")
                    nc.vector.scalar_tensor_tensor(pn[:], pg[0][:], sc_p, p[uc][:],
                                                   ALU.mult, ALU.add)
                    p[uc] = pn
                    if first:
                        qn = stp.tile([F, Bc], F32, name=f"qn{uc}{tg}", tag=f"q{uc}")
                        nc.vector.scalar_tensor_tensor(qn[:], pg[1][:], sc_q,
                                                       q[uc][:], ALU.mult, ALU.add)
                        q[uc] = qn

            for step in range(STEPS_HALF):
                for ev in range(2):
                    if step == 0 and ev == 0:
                        # both chains share the initial state: one gradient
                        # eval, four updates
                        emit_eval(0, first=True, ev_id=0, update_chains=(0, 1))
                        continue
                    ph2 = [None, None]
                    for c in range(2):
                        ph2[c] = _emit_phase1(c, ev == 0, 2 * step + ev, (c,))
                    for c in range(2):
                        ph2[c]()

            nc.sync.dma_start(out=outs_d["oqf"][:], in_=q[0][:])
            nc.sync.dma_start(out=outs_d["opf"][:], in_=p[0][:])
            nc.sync.dma_start(out=outs_d["oqb"][:], in_=q[1][:])
            nc.sync.dma_start(out=outs_d["opb"][:], in_=p[1][:])

    nc.finalize()
    return nc


_NC_CACHE = {}


def _get_nc():
    if "nc" not in _NC_CACHE:
        _NC_CACHE["nc"] = _build_program()
    return _NC_CACHE["nc"]


def _blk(w, dtype=ml_dtypes.bfloat16):
    """[256, 256] -> [128, 2, 256] with blk[p, kc, m] = w[kc*128 + p, m]."""
    return np.ascontiguousarray(w.reshape(2, F, 2 * F).transpose(1, 0, 2)).astype(dtype)


def _col2(v):
    """[256] -> [128, 2] with out[p, jc] = v[jc*128 + p]."""
    return np.ascontiguousarray(v.reshape(2, F).T.astype(np.float32))


def _prepare_in_maps(x, W1, b1, W2, b2, Wout):
    x = np.asarray(x, np.float32)
    W1 = np.asarray(W1, np.float32)
    W2 = np.asarray(W2, np.float32)
    wout = np.asarray(Wout, np.float32).reshape(-1)
    b1 = np.asarray(b1, np.float32).reshape(-1)
    b2 = np.asarray(b2, np.float32).reshape(-1)

    q_mid = x[:, MID, :]                       # [B, F]
    p_mid = q_mid - x[:, MID - 1, :]
    qt = np.ascontiguousarray(q_mid.T)         # [F, B]
    pt = np.ascontiguousarray(p_mid.T)

    w2tw = (W2.T * wout[:, None]).astype(np.float32)  # [j,i] = wout[j]*W2[i,j]
    # c2 must match the fp16 weights actually used in the matmul
    c2 = _blk(w2tw).astype(np.float32).transpose(1, 0, 2).reshape(2 * F, 2 * F).sum(axis=0)

    shared = {
        "w1": _blk(W1), "w2": _blk(W2), "w2tw": _blk(w2tw),
        "w1t": _blk(np.ascontiguousarray(W1.T)),
        "b1c": _col2(b1), "b2c": _col2(b2), "c2c": _col2(c2),
    }
    in_maps = []
    for core in range(N_CORES):
        sl = slice(core * Bc, (core + 1) * Bc)
        m = dict(shared)
        m["qt"] = np.ascontiguousarray(qt[:, sl])
        m["pt"] = np.ascontiguousarray(pt[:, sl])
        m["qt16"] = m["qt"].astype(ml_dtypes.bfloat16)
        m["pt16"] = m["pt"].astype(ml_dtypes.bfloat16)
        in_maps.append(m)
    return in_maps, q_mid, p_mid


def _assemble(results, q_mid, p_mid):
    out = np.empty((B, 6 * F), np.float32)
    out[:, 2 * F:3 * F] = q_mid
    out[:, 3 * F:4 * F] = p_mid
    for core in range(N_CORES):
        sl = slice(core * Bc, (core + 1) * Bc)
        r = results[core]
        out[sl, 0:F] = r["oqb"].T
        out[sl, F:2 * F] = r["opb"].T
        out[sl, 4 * F:5 * F] = r["oqf"].T
        out[sl, 5 * F:6 * F] = r["opf"].T
    return out


def run(trace=False, **inputs):
    """Full pipeline; returns (output, BassKernelResults)."""
    in_maps, q_mid, p_mid = _prepare_in_maps(**inputs)
    nc = _get_nc()
    res = run_bass_kernel_spmd(nc, in_maps, list(range(N_CORES)), trace=trace)
    return _assemble(res.results, q_mid, p_mid), res


def kernel(**inputs) -> np.ndarray:
    out, _ = run(trace=False, **inputs)
    return out

